# revision 23
# baseline (speedup 1.0000x reference)
"""Causal GQA cross-attention kernel for Trainium2, 8-core SPMD.

Problem: q [2, 2048, 16, 128] f32, kv [2, 2048, 2, 8, 128] f32 ->
out [2, 2048, 16, 128] f32; causal mask (Sq == Sk), GQA with 2 q heads
per kv head, softmax scale 1/sqrt(128).

Sharding: 2 batches x 4 kv-head-pairs -> 8 cores. Each core gets 4 q
heads + 2 kv heads (its GQA groups), computes attention locally; no
collectives. Host splits/gathers.

v3: host-side transposes/casts (device gets qT/kT [.., D, S] bf16 and
v [.., P, kb, D] bf16, loaded with large plain DMAs). Scores computed
as S^T[k, q] = K^T.T @ Q^T per 512-q superblock. Off-diagonal k-block
pairs go through full [128,1024] PSUM tiles with no masking needed;
the four diagonal k-blocks are PACKED: only the causally-valid q strip
of each block is computed/exp'd (512+384+128 cols in one tile, 256 in
a second), cutting ~15% of Tensor and ACT columns and shrinking the
mask multiplies to the diagonal strips only.

P^T = exp(S^T * scale) on ACT, out bf16. PV: out[q, d|denom] +=
(P^T block).T @ [V | ones] (PSUM f32 accumulate over k blocks); the
ones column yields the softmax denominator free. Store unnormalized
[q, 129] per 512-q superblock; host divides by column 128.
"""

import math
import os
import sys

import numpy as np
import ml_dtypes

sys.path.insert(0, "/opt/trn_rl_repo")

import concourse.bass as bass  # noqa: E402
import concourse.mybir as mybir  # noqa: E402
import concourse.tile as tile  # noqa: E402
from concourse import bacc  # noqa: E402
from concourse.bass_utils import run_bass_kernel_spmd  # noqa: E402

B, SQ, SK, H, HKV, D = 2, 2048, 2048, 16, 8, 128
NCORES = 8
NQH = H * B // NCORES  # 4 q heads per core
NKVH = HKV * B // NCORES  # 2 kv heads per core
P = 128
NQB = SQ // P  # 16 q blocks of 128
NSB = 4  # q superblocks of 512
SBW = 512
NKB = SK // P  # 16 k blocks
SCALE = 1.0 / math.sqrt(D)

F32 = mybir.dt.float32
BF16 = mybir.dt.bfloat16

# diagonal strips in tile A: (dk, col offset, width, q offset in superblock)
DIAG_A = ((0, 0, 512, 0), (1, 512, 384, 128), (3, 896, 128, 384))
# strip in tile B
DIAG_B = (2, 0, 256, 256)

LAST_RESULTS = None
_CACHE = {}


def build_module():
    nc = bacc.Bacc(None, target_bir_lowering=False)

    qt_d = nc.dram_tensor("qt", [NQH, D, SQ], BF16, kind="ExternalInput")
    kt_d = nc.dram_tensor("kt", [NKVH, D, SK], BF16, kind="ExternalInput")
    v_d = nc.dram_tensor("v", [NKVH, P, NKB, D], BF16, kind="ExternalInput")
    o_d = nc.dram_tensor("o", [NQH, NQB, P, D + 1], F32, kind="ExternalOutput")

    with tile.TileContext(nc) as tc:
        with (
            tc.tile_pool(name="const", bufs=1) as constp,
            tc.tile_pool(name="kt", bufs=2) as ktp,
            tc.tile_pool(name="qt", bufs=2) as qtp,
            tc.tile_pool(name="vaug", bufs=2) as vap,
            tc.tile_pool(name="pt", bufs=12) as ptp,
            tc.tile_pool(name="outs", bufs=3) as outp,
            tc.tile_pool(name="pst", bufs=3, space="PSUM") as pstp,
            tc.tile_pool(name="ppv", bufs=2, space="PSUM") as ppvp,
        ):
            # dummy exp on a 1-col tile: preloads the ACT exp table while
            # the first input DMAs are in flight
            warm = constp.tile([P, 2], F32, tag="warm")
            nc.gpsimd.memset(warm[:, 0:1], 0.0)
            nc.scalar.activation(
                warm[:, 1:2],
                warm[:, 0:1],
                mybir.ActivationFunctionType.Exp,
                scale=SCALE,
            )

            # diagonal strip masks: within each strip, col c (rel) is valid
            # iff c >= k (partition). Strips start at their diagonal.
            mask_a = constp.tile([P, 2 * SBW], BF16, tag="maskA")
            nc.gpsimd.memset(mask_a[:], 1.0)
            for _, off, w, _ in DIAG_A:
                nc.gpsimd.affine_select(
                    out=mask_a[:, off : off + w],
                    in_=mask_a[:, off : off + w],
                    compare_op=mybir.AluOpType.is_ge,
                    fill=0.0,
                    base=0,
                    pattern=[[1, w]],
                    channel_multiplier=-1,
                )
            mask_b = constp.tile([P, 256], BF16, tag="maskB")
            nc.gpsimd.memset(mask_b[:], 1.0)
            nc.gpsimd.affine_select(
                out=mask_b[:],
                in_=mask_b[:],
                compare_op=mybir.AluOpType.is_ge,
                fill=0.0,
                base=0,
                pattern=[[1, 256]],
                channel_multiplier=-1,
            )

            def pv_block(pvs, kb, j, src, qb_hi):
                # accumulate pvs[j] += src.T @ vaug for q block j
                nc.tensor.matmul(
                    pvs[j],
                    src,
                    vaug_cur[0][:, kb, :],
                    start=(kb == 0),
                    stop=(kb == qb_hi),
                )

            vaug_cur = [None]

            def head_compute(h, qt, kt_g, sb_order):
                for sb in sb_order:
                    # PV accumulators: j=0,1 live through the pair loop in 2
                    # PSUM banks (a start_tensor_calc claims a whole 2KB zero
                    # region, so groups can't share banks); j=2,3 run as a
                    # deferred burst afterwards, reusing the same 2 banks and
                    # the kept P^T tiles.
                    pvs = {
                        j: ppvp.tile(
                            [P, D + 1], F32, tag="ppv", name=f"pv_{h}_{sb}_{j}"
                        )[:]
                        for j in (0, 1)
                    }
                    kb_src = {}  # kb -> (tile, base col) for P^T slices
                    # off-diagonal pairs: kb = 2*pair, 2*pair+1, all < 4*sb
                    for pair in range(2 * sb):
                        st = pstp.tile([P, 2 * SBW], F32, tag="pst")
                        for half in (0, 1):
                            kb = 2 * pair + half
                            nc.tensor.matmul(
                                st[:, half * SBW : (half + 1) * SBW],
                                kt_g[:, kb * P : (kb + 1) * P],
                                qt[:, sb * SBW : (sb + 1) * SBW],
                                start=True,
                                stop=True,
                            )
                        pt = ptp.tile([P, 2 * SBW], BF16, tag="pt")
                        nc.scalar.activation(
                            pt[:],
                            st[:],
                            mybir.ActivationFunctionType.Exp,
                            scale=SCALE,
                        )
                        for half in (0, 1):
                            kb = 2 * pair + half
                            kb_src[kb] = (pt, half * SBW)
                            for j in (0, 1):
                                pv_block(
                                    pvs,
                                    kb,
                                    j,
                                    pt[:, half * SBW + j * P : half * SBW + (j + 1) * P],
                                    4 * sb + j,
                                )
                    # diagonal blocks kb0..kb0+3, packed valid strips
                    kb0 = 4 * sb
                    st_a = pstp.tile([P, 2 * SBW], F32, tag="pst")
                    st_b = pstp.tile([P, 2 * SBW], F32, tag="pst")
                    for dk, off, w, qoff in DIAG_A:
                        nc.tensor.matmul(
                            st_a[:, off : off + w],
                            kt_g[:, (kb0 + dk) * P : (kb0 + dk + 1) * P],
                            qt[:, sb * SBW + qoff : (sb + 1) * SBW],
                            start=True,
                            stop=True,
                        )
                    dkb, boff, bw, bqoff = DIAG_B
                    nc.tensor.matmul(
                        st_b[:, boff : boff + bw],
                        kt_g[:, (kb0 + dkb) * P : (kb0 + dkb + 1) * P],
                        qt[:, sb * SBW + bqoff : (sb + 1) * SBW],
                        start=True,
                        stop=True,
                    )
                    pt_a = ptp.tile([P, 2 * SBW], BF16, tag="pt")
                    nc.scalar.activation(
                        pt_a[:],
                        st_a[:],
                        mybir.ActivationFunctionType.Exp,
                        scale=SCALE,
                    )
                    nc.vector.tensor_tensor(
                        out=pt_a[:],
                        in0=pt_a[:],
                        in1=mask_a[:],
                        op=mybir.AluOpType.mult,
                    )
                    pt_b = ptp.tile([P, 2 * SBW], BF16, tag="pt")
                    nc.scalar.activation(
                        pt_b[:, 0:256],
                        st_b[:, 0:256],
                        mybir.ActivationFunctionType.Exp,
                        scale=SCALE,
                    )
                    nc.vector.tensor_tensor(
                        out=pt_b[:, 0:256],
                        in0=pt_b[:, 0:256],
                        in1=mask_b[:],
                        op=mybir.AluOpType.mult,
                    )
                    for dk in range(4):
                        kb = kb0 + dk
                        if dk == DIAG_B[0]:
                            _, off, w, _ = DIAG_B
                            src_t, base = pt_b, off - dk * P
                        else:
                            off, w = next(
                                (o, ww) for d, o, ww, _ in DIAG_A if d == dk
                            )
                            src_t, base = pt_a, off - dk * P
                        kb_src[kb] = (src_t, base)
                        for j in (0, 1):
                            if dk <= j:
                                pv_block(
                                    pvs,
                                    kb,
                                    j,
                                    src_t[:, base + j * P : base + (j + 1) * P],
                                    4 * sb + j,
                                )
                    ot = outp.tile([P, 4, D + 1], F32, tag="outs")
                    for j in (0, 1):
                        nc.vector.tensor_copy(ot[:, j, :], pvs[j])
                    nc.sync.dma_start(
                        o_d[h, 4 * sb : 4 * sb + 2].rearrange("qb p d -> p qb d"),
                        ot[:, 0:2, :],
                    )
                    # deferred j=2,3 accumulation burst over kept P^T tiles
                    for j in (2, 3):
                        pvs[j] = ppvp.tile(
                            [P, D + 1], F32, tag="ppv", name=f"pv_{h}_{sb}_{j}"
                        )[:]
                    for kb in range(kb0 + 4):
                        src_t, base = kb_src[kb]
                        for j in (2, 3):
                            if kb - kb0 <= j:
                                pv_block(
                                    pvs,
                                    kb,
                                    j,
                                    src_t[:, base + j * P : base + (j + 1) * P],
                                    4 * sb + j,
                                )
                    for j in (2, 3):
                        nc.vector.tensor_copy(ot[:, j, :], pvs[j])
                    if h == 3 and sb == sb_order[-1]:
                        # final stores are on the kernel's critical tail:
                        # split across queues
                        for j in (2, 3):
                            nc.sync.dma_start(o_d[h, 4 * sb + j], ot[:, j, :])
                    else:
                        nc.sync.dma_start(
                            o_d[h, 4 * sb + 2 : 4 * sb + 4].rearrange(
                                "qb p d -> p qb d"
                            ),
                            ot[:, 2:4, :],
                        )

            NCH = 4  # DMA chunks per kt/qt load (parallel queues)
            CW = SK // NCH

            def load_kt(kt_g, g, chunks):
                for ch in chunks:
                    nc.sync.dma_start(
                        kt_g[:, ch * CW : (ch + 1) * CW],
                        kt_d[g, :, ch * CW : (ch + 1) * CW],
                    )

            def load_qt(qt, h, chunks):
                for ch in chunks:
                    nc.sync.dma_start(
                        qt[:, ch * CW : (ch + 1) * CW],
                        qt_d[h, :, ch * CW : (ch + 1) * CW],
                    )

            for g in range(NKVH):
                kt_g = ktp.tile([P, SK], BF16, tag="kt")
                qt0 = qtp.tile([P, SQ], BF16, tag="qt")
                if g == 0:
                    # dispatch order follows head 0's sb order [0, 3, 2, 1]:
                    # sb=0 reads kt[:, 0:512] / qt[:, 0:512]; sb=3 then needs
                    # all kt chunks plus qt chunk 3
                    load_kt(kt_g, g, [0])
                    load_qt(qt0, 2 * g, [0])
                    load_kt(kt_g, g, [1, 2, 3])
                    load_qt(qt0, 2 * g, [3, 1, 2])
                else:
                    load_kt(kt_g, g, range(NCH))
                    load_qt(qt0, 2 * g, range(NCH))
                vaug_g = vap.tile([P, NKB, D + 1], BF16, tag="vaug")
                nc.gpsimd.memset(vaug_g[:, :, D : D + 1], 1.0)
                for ch in range(2):
                    nc.sync.dma_start(
                        vaug_g[:, ch * NKB // 2 : (ch + 1) * NKB // 2, 0:D],
                        v_d[g, :, ch * NKB // 2 : (ch + 1) * NKB // 2],
                    )
                vaug_cur[0] = vaug_g
                for hl in range(2):
                    h = 2 * g + hl
                    if hl == 0:
                        qt = qt0
                    else:
                        qt = qtp.tile([P, SQ], BF16, tag="qt")
                        load_qt(qt, h, range(NCH))
                    # end every head on a small sb so its deferred PV burst
                    # (which stalls the next head's scores on Tensor) is the
                    # smallest one; head 0 starts with sb=0 so only the first
                    # qt/kt chunks gate the pipeline start
                    sb_order = [0, 3, 2, 1] if h == 0 else [3, 2, 1, 0]
                    head_compute(h, qt[:], kt_g[:], sb_order)

    nc.finalize()
    return nc


def _get_module():
    if "nc" not in _CACHE:
        _CACHE["nc"] = build_module()
    return _CACHE["nc"]


def kernel(q, kv):
    global LAST_RESULTS
    q = np.asarray(q, dtype=np.float32)
    kv = np.asarray(kv, dtype=np.float32)

    nc = _get_module()
    in_maps = []
    for c in range(NCORES):
        b, j = divmod(c, 4)
        # qT: [Sq, 4, D] -> [4, D, Sq]
        q_s = np.ascontiguousarray(
            np.transpose(q[b][:, 4 * j : 4 * j + 4, :], (1, 2, 0))
        ).astype(ml_dtypes.bfloat16)
        # kT: [Sk, 2, D] -> [2, D, Sk]
        k_s = np.ascontiguousarray(
            np.transpose(kv[b][:, 0, 2 * j : 2 * j + 2, :], (1, 2, 0))
        ).astype(ml_dtypes.bfloat16)
        # v: [Sk, 2, D] -> [2, P, NKB, D]
        v_s = np.ascontiguousarray(
            kv[b][:, 1, 2 * j : 2 * j + 2, :]
            .reshape(NKB, P, NKVH, D)
            .transpose(2, 1, 0, 3)
        ).astype(ml_dtypes.bfloat16)
        in_maps.append({"qt": q_s, "kt": k_s, "v": v_s})

    trace = bool(int(os.environ.get("KERNEL_TRACE", "0")))
    kwargs = {}
    tdir = os.environ.get("KERNEL_TRACE_DIR")
    if tdir:
        kwargs["tmpdir"] = tdir
    res = run_bass_kernel_spmd(
        nc, in_maps, core_ids=list(range(NCORES)), trace=trace, **kwargs
    )
    LAST_RESULTS = res

    out = np.empty((B, SQ, H, D), np.float32)
    for c in range(NCORES):
        b, j = divmod(c, 4)
        o = res.results[c]["o"].reshape(NQH, SQ, D + 1)
        norm = o[..., :D] / o[..., D : D + 1]
        out[b, :, 4 * j : 4 * j + 4, :] = np.transpose(norm, (1, 0, 2))
    return out


# revision 27
# speedup vs baseline: 1.1321x; 1.1321x over previous
"""Causal GQA cross-attention kernel for Trainium2, 8-core SPMD.

Problem: q [2, 2048, 16, 128] f32, kv [2, 2048, 2, 8, 128] f32 ->
out [2, 2048, 16, 128] f32; causal mask (Sq == Sk), GQA with 2 q heads
per kv head, softmax scale 1/sqrt(128).

Sharding: 2 batches x 4 kv-head-pairs -> 8 cores. Each core gets 4 q
heads + 2 kv heads (its GQA groups), computes attention locally; no
collectives. Host splits/gathers.

v3: host-side transposes/casts (device gets qT/kT [.., D, S] bf16 and
v [.., P, kb, D] bf16, loaded with large plain DMAs). Scores computed
as S^T[k, q] = K^T.T @ Q^T per 512-q superblock. Off-diagonal k-block
pairs go through full [128,1024] PSUM tiles with no masking needed;
the four diagonal k-blocks are PACKED: only the causally-valid q strip
of each block is computed/exp'd (512+384+128 cols in one tile, 256 in
a second), cutting ~15% of Tensor and ACT columns and shrinking the
mask multiplies to the diagonal strips only.

P^T = exp(S^T * scale) on ACT, out bf16. PV: out[q, d|denom] +=
(P^T block).T @ [V | ones] (PSUM f32 accumulate over k blocks); the
ones column yields the softmax denominator free. Store unnormalized
[q, 129] per 512-q superblock; host divides by column 128.
"""

import math
import os
import sys

import numpy as np
import ml_dtypes

sys.path.insert(0, "/opt/trn_rl_repo")

import concourse.bass as bass  # noqa: E402
import concourse.mybir as mybir  # noqa: E402
import concourse.tile as tile  # noqa: E402
from concourse import bacc  # noqa: E402
from concourse.bass_utils import run_bass_kernel_spmd  # noqa: E402

B, SQ, SK, H, HKV, D = 2, 2048, 2048, 16, 8, 128
NCORES = 8
NQH = H * B // NCORES  # 4 q heads per core
NKVH = HKV * B // NCORES  # 2 kv heads per core
P = 128
NQB = SQ // P  # 16 q blocks of 128
NSB = 4  # q superblocks of 512
SBW = 512
NKB = SK // P  # 16 k blocks
SCALE = 1.0 / math.sqrt(D)

F32 = mybir.dt.float32
BF16 = mybir.dt.bfloat16

# diagonal strips in tile A: (dk, col offset, width, q offset in superblock)
DIAG_A = ((0, 0, 512, 0), (1, 512, 384, 128), (3, 896, 128, 384))
# strip in tile B
DIAG_B = (2, 0, 256, 256)

LAST_RESULTS = None
_CACHE = {}


def build_module():
    nc = bacc.Bacc(None, target_bir_lowering=False)

    qt_d = nc.dram_tensor("qt", [NQH, D, SQ], BF16, kind="ExternalInput")
    kt_d = nc.dram_tensor("kt", [NKVH, D, SK], BF16, kind="ExternalInput")
    v_d = nc.dram_tensor("v", [NKVH, P, NKB, D], BF16, kind="ExternalInput")
    o_d = nc.dram_tensor("o", [NQH, NQB, P, D + 1], F32, kind="ExternalOutput")

    with tile.TileContext(nc) as tc:
        with (
            tc.tile_pool(name="const", bufs=1) as constp,
            tc.tile_pool(name="kt", bufs=2) as ktp,
            tc.tile_pool(name="qt", bufs=2) as qtp,
            tc.tile_pool(name="vaug", bufs=2) as vap,
            tc.tile_pool(name="pt", bufs=12) as ptp,
            tc.tile_pool(name="outs", bufs=3) as outp,
            tc.tile_pool(name="pst", bufs=3, space="PSUM") as pstp,
            tc.tile_pool(name="ppv", bufs=2, space="PSUM") as ppvp,
        ):
            # dummy exp on a 1-col tile: preloads the ACT exp table while
            # the first input DMAs are in flight
            warm = constp.tile([P, 2], F32, tag="warm")
            nc.gpsimd.memset(warm[:, 0:1], 0.0)
            nc.scalar.activation(
                warm[:, 1:2],
                warm[:, 0:1],
                mybir.ActivationFunctionType.Exp,
                scale=SCALE,
            )

            # diagonal strip masks: within each strip, col c (rel) is valid
            # iff c >= k (partition). Strips start at their diagonal.
            mask_a = constp.tile([P, 2 * SBW], BF16, tag="maskA")
            nc.gpsimd.memset(mask_a[:], 1.0)
            for _, off, w, _ in DIAG_A:
                nc.gpsimd.affine_select(
                    out=mask_a[:, off : off + w],
                    in_=mask_a[:, off : off + w],
                    compare_op=mybir.AluOpType.is_ge,
                    fill=0.0,
                    base=0,
                    pattern=[[1, w]],
                    channel_multiplier=-1,
                )
            mask_b = constp.tile([P, 256], BF16, tag="maskB")
            nc.gpsimd.memset(mask_b[:], 1.0)
            nc.gpsimd.affine_select(
                out=mask_b[:],
                in_=mask_b[:],
                compare_op=mybir.AluOpType.is_ge,
                fill=0.0,
                base=0,
                pattern=[[1, 256]],
                channel_multiplier=-1,
            )

            def pv_block(pvs, kb, j, src, qb_hi):
                # accumulate pvs[j] += src.T @ vaug for q block j
                nc.tensor.matmul(
                    pvs[j],
                    src,
                    vaug_cur[0][:, kb, :],
                    start=(kb == 0),
                    stop=(kb == qb_hi),
                )

            vaug_cur = [None]

            def head_compute(h, qt, kt_g, sb_order):
                for sb in sb_order:
                    # PV accumulators: j=0,1 live through the pair loop in 2
                    # PSUM banks (a start_tensor_calc claims a whole 2KB zero
                    # region, so groups can't share banks); j=2,3 run as a
                    # deferred burst afterwards, reusing the same 2 banks and
                    # the kept P^T tiles.
                    pvs = {
                        j: ppvp.tile(
                            [P, D + 1], F32, tag="ppv", name=f"pv_{h}_{sb}_{j}"
                        )[:]
                        for j in (0, 1)
                    }
                    kb_src = {}  # kb -> (tile, base col) for P^T slices
                    # off-diagonal pairs: kb = 2*pair, 2*pair+1, all < 4*sb
                    for pair in range(2 * sb):
                        st = pstp.tile([P, 2 * SBW], F32, tag="pst")
                        for half in (0, 1):
                            kb = 2 * pair + half
                            nc.tensor.matmul(
                                st[:, half * SBW : (half + 1) * SBW],
                                kt_g[:, kb * P : (kb + 1) * P],
                                qt[:, sb * SBW : (sb + 1) * SBW],
                                start=True,
                                stop=True,
                            )
                        pt = ptp.tile([P, 2 * SBW], BF16, tag="pt")
                        nc.scalar.activation(
                            pt[:],
                            st[:],
                            mybir.ActivationFunctionType.Exp,
                            scale=SCALE,
                        )
                        for half in (0, 1):
                            kb = 2 * pair + half
                            kb_src[kb] = (pt, half * SBW)
                            for j in (0, 1):
                                pv_block(
                                    pvs,
                                    kb,
                                    j,
                                    pt[:, half * SBW + j * P : half * SBW + (j + 1) * P],
                                    4 * sb + j,
                                )
                    # diagonal blocks kb0..kb0+3, packed valid strips
                    kb0 = 4 * sb
                    st_a = pstp.tile([P, 2 * SBW], F32, tag="pst")
                    st_b = pstp.tile([P, 2 * SBW], F32, tag="pst")
                    for dk, off, w, qoff in DIAG_A:
                        nc.tensor.matmul(
                            st_a[:, off : off + w],
                            kt_g[:, (kb0 + dk) * P : (kb0 + dk + 1) * P],
                            qt[:, sb * SBW + qoff : (sb + 1) * SBW],
                            start=True,
                            stop=True,
                        )
                    dkb, boff, bw, bqoff = DIAG_B
                    nc.tensor.matmul(
                        st_b[:, boff : boff + bw],
                        kt_g[:, (kb0 + dkb) * P : (kb0 + dkb + 1) * P],
                        qt[:, sb * SBW + bqoff : (sb + 1) * SBW],
                        start=True,
                        stop=True,
                    )
                    pt_a = ptp.tile([P, 2 * SBW], BF16, tag="pt")
                    nc.scalar.activation(
                        pt_a[:],
                        st_a[:],
                        mybir.ActivationFunctionType.Exp,
                        scale=SCALE,
                    )
                    nc.vector.tensor_tensor(
                        out=pt_a[:],
                        in0=pt_a[:],
                        in1=mask_a[:],
                        op=mybir.AluOpType.mult,
                    )
                    pt_b = ptp.tile([P, 2 * SBW], BF16, tag="pt")
                    nc.scalar.activation(
                        pt_b[:, 0:256],
                        st_b[:, 0:256],
                        mybir.ActivationFunctionType.Exp,
                        scale=SCALE,
                    )
                    nc.vector.tensor_tensor(
                        out=pt_b[:, 0:256],
                        in0=pt_b[:, 0:256],
                        in1=mask_b[:],
                        op=mybir.AluOpType.mult,
                    )
                    for dk in range(4):
                        kb = kb0 + dk
                        if dk == DIAG_B[0]:
                            _, off, w, _ = DIAG_B
                            src_t, base = pt_b, off - dk * P
                        else:
                            off, w = next(
                                (o, ww) for d, o, ww, _ in DIAG_A if d == dk
                            )
                            src_t, base = pt_a, off - dk * P
                        kb_src[kb] = (src_t, base)
                        for j in (0, 1):
                            if dk <= j:
                                pv_block(
                                    pvs,
                                    kb,
                                    j,
                                    src_t[:, base + j * P : base + (j + 1) * P],
                                    4 * sb + j,
                                )
                    ot = outp.tile([P, 4, D + 1], F32, tag="outs")
                    for j in (0, 1):
                        nc.vector.tensor_copy(ot[:, j, :], pvs[j])
                    nc.sync.dma_start(
                        o_d[h, 4 * sb : 4 * sb + 2].rearrange("qb p d -> p qb d"),
                        ot[:, 0:2, :],
                    )
                    # deferred j=2,3 accumulation burst over kept P^T tiles
                    for j in (2, 3):
                        pvs[j] = ppvp.tile(
                            [P, D + 1], F32, tag="ppv", name=f"pv_{h}_{sb}_{j}"
                        )[:]
                    for kb in range(kb0 + 4):
                        src_t, base = kb_src[kb]
                        for j in (2, 3):
                            if kb - kb0 <= j:
                                pv_block(
                                    pvs,
                                    kb,
                                    j,
                                    src_t[:, base + j * P : base + (j + 1) * P],
                                    4 * sb + j,
                                )
                    for j in (2, 3):
                        nc.vector.tensor_copy(ot[:, j, :], pvs[j])
                    nc.sync.dma_start(
                        o_d[h, 4 * sb + 2 : 4 * sb + 4].rearrange("qb p d -> p qb d"),
                        ot[:, 2:4, :],
                    )

            NCH = 4  # DMA chunks per kt/qt load (parallel queues)
            CW = SK // NCH

            def load_kt(kt_g, g, chunks):
                for ch in chunks:
                    nc.sync.dma_start(
                        kt_g[:, ch * CW : (ch + 1) * CW],
                        kt_d[g, :, ch * CW : (ch + 1) * CW],
                    )

            def load_qt(qt, h, chunks):
                for ch in chunks:
                    nc.sync.dma_start(
                        qt[:, ch * CW : (ch + 1) * CW],
                        qt_d[h, :, ch * CW : (ch + 1) * CW],
                    )

            for g in range(NKVH):
                kt_g = ktp.tile([P, SK], BF16, tag="kt")
                qt0 = qtp.tile([P, SQ], BF16, tag="qt")
                if g == 0:
                    # first-needed chunks first: the very first score tile
                    # reads kt[:, 0:512] and qt[:, 0:512]
                    load_kt(kt_g, g, [0])
                    load_qt(qt0, 2 * g, [0, 1])
                    load_kt(kt_g, g, [1])
                    load_qt(qt0, 2 * g, [2, 3])
                    load_kt(kt_g, g, [2, 3])
                else:
                    load_kt(kt_g, g, range(NCH))
                    load_qt(qt0, 2 * g, range(NCH))
                vaug_g = vap.tile([P, NKB, D + 1], BF16, tag="vaug")
                nc.gpsimd.memset(vaug_g[:, :, D : D + 1], 1.0)
                for ch in range(2):
                    nc.sync.dma_start(
                        vaug_g[:, ch * NKB // 2 : (ch + 1) * NKB // 2, 0:D],
                        v_d[g, :, ch * NKB // 2 : (ch + 1) * NKB // 2],
                    )
                vaug_cur[0] = vaug_g
                for hl in range(2):
                    h = 2 * g + hl
                    if hl == 0:
                        qt = qt0
                    else:
                        qt = qtp.tile([P, SQ], BF16, tag="qt")
                        load_qt(qt, h, range(NCH))
                    # end every head on a small sb so its deferred PV burst
                    # (which stalls the next head's scores on Tensor) is the
                    # smallest one; head 0 starts with sb=0 so only the first
                    # qt/kt chunks gate the pipeline start
                    sb_order = [0, 3, 2, 1] if h == 0 else [3, 2, 1, 0]
                    head_compute(h, qt[:], kt_g[:], sb_order)

    nc.finalize()
    return nc


def _get_module():
    if "nc" not in _CACHE:
        _CACHE["nc"] = build_module()
    return _CACHE["nc"]


def kernel(q, kv):
    global LAST_RESULTS
    q = np.asarray(q, dtype=np.float32)
    kv = np.asarray(kv, dtype=np.float32)

    nc = _get_module()
    in_maps = []
    for c in range(NCORES):
        b, j = divmod(c, 4)
        # qT: [Sq, 4, D] -> [4, D, Sq]
        q_s = np.ascontiguousarray(
            np.transpose(q[b][:, 4 * j : 4 * j + 4, :], (1, 2, 0))
        ).astype(ml_dtypes.bfloat16)
        # kT: [Sk, 2, D] -> [2, D, Sk]
        k_s = np.ascontiguousarray(
            np.transpose(kv[b][:, 0, 2 * j : 2 * j + 2, :], (1, 2, 0))
        ).astype(ml_dtypes.bfloat16)
        # v: [Sk, 2, D] -> [2, P, NKB, D]
        v_s = np.ascontiguousarray(
            kv[b][:, 1, 2 * j : 2 * j + 2, :]
            .reshape(NKB, P, NKVH, D)
            .transpose(2, 1, 0, 3)
        ).astype(ml_dtypes.bfloat16)
        in_maps.append({"qt": q_s, "kt": k_s, "v": v_s})

    trace = bool(int(os.environ.get("KERNEL_TRACE", "0")))
    kwargs = {}
    tdir = os.environ.get("KERNEL_TRACE_DIR")
    if tdir:
        kwargs["tmpdir"] = tdir
    res = run_bass_kernel_spmd(
        nc, in_maps, core_ids=list(range(NCORES)), trace=trace, **kwargs
    )
    LAST_RESULTS = res

    out = np.empty((B, SQ, H, D), np.float32)
    for c in range(NCORES):
        b, j = divmod(c, 4)
        o = res.results[c]["o"].reshape(NQH, SQ, D + 1)
        norm = o[..., :D] / o[..., D : D + 1]
        out[b, :, 4 * j : 4 * j + 4, :] = np.transpose(norm, (1, 0, 2))
    return out


# revision 28
# speedup vs baseline: 1.1892x; 1.0504x over previous
"""Causal GQA cross-attention kernel for Trainium2, 8-core SPMD.

Problem: q [2, 2048, 16, 128] f32, kv [2, 2048, 2, 8, 128] f32 ->
out [2, 2048, 16, 128] f32; causal mask (Sq == Sk), GQA with 2 q heads
per kv head, softmax scale 1/sqrt(128).

Sharding: 2 batches x 4 kv-head-pairs -> 8 cores. Each core gets 4 q
heads + 2 kv heads (its GQA groups), computes attention locally; no
collectives. Host splits/gathers.

v3: host-side transposes/casts (device gets qT/kT [.., D, S] bf16 and
v [.., P, kb, D] bf16, loaded with large plain DMAs). Scores computed
as S^T[k, q] = K^T.T @ Q^T per 512-q superblock. Off-diagonal k-block
pairs go through full [128,1024] PSUM tiles with no masking needed;
the four diagonal k-blocks are PACKED: only the causally-valid q strip
of each block is computed/exp'd (512+384+128 cols in one tile, 256 in
a second), cutting ~15% of Tensor and ACT columns and shrinking the
mask multiplies to the diagonal strips only.

P^T = exp(S^T * scale) on ACT, out bf16. PV: out[q, d|denom] +=
(P^T block).T @ [V | ones] (PSUM f32 accumulate over k blocks); the
ones column yields the softmax denominator free. Store unnormalized
[q, 129] per 512-q superblock; host divides by column 128.
"""

import math
import os
import sys

import numpy as np
import ml_dtypes

sys.path.insert(0, "/opt/trn_rl_repo")

import concourse.bass as bass  # noqa: E402
import concourse.mybir as mybir  # noqa: E402
import concourse.tile as tile  # noqa: E402
from concourse import bacc  # noqa: E402
from concourse.bass_utils import run_bass_kernel_spmd  # noqa: E402

B, SQ, SK, H, HKV, D = 2, 2048, 2048, 16, 8, 128
NCORES = 8
NQH = H * B // NCORES  # 4 q heads per core
NKVH = HKV * B // NCORES  # 2 kv heads per core
P = 128
NQB = SQ // P  # 16 q blocks of 128
NSB = 4  # q superblocks of 512
SBW = 512
NKB = SK // P  # 16 k blocks
SCALE = 1.0 / math.sqrt(D)

F32 = mybir.dt.float32
BF16 = mybir.dt.bfloat16

# diagonal strips in tile A: (dk, col offset, width, q offset in superblock)
DIAG_A = ((0, 0, 512, 0), (1, 512, 384, 128), (3, 896, 128, 384))
# strip in tile B
DIAG_B = (2, 0, 256, 256)

LAST_RESULTS = None
_CACHE = {}


def build_module():
    nc = bacc.Bacc(None, target_bir_lowering=False)

    qt_d = nc.dram_tensor("qt", [NQH, D, SQ], BF16, kind="ExternalInput")
    kt_d = nc.dram_tensor("kt", [NKVH, D, SK], BF16, kind="ExternalInput")
    v_d = nc.dram_tensor("v", [NKVH, P, NKB, D], BF16, kind="ExternalInput")
    o_d = nc.dram_tensor("o", [NQH, NQB, P, D + 1], F32, kind="ExternalOutput")

    with tile.TileContext(nc) as tc:
        with (
            tc.tile_pool(name="const", bufs=1) as constp,
            tc.tile_pool(name="kt", bufs=2) as ktp,
            tc.tile_pool(name="qt", bufs=2) as qtp,
            tc.tile_pool(name="vaug", bufs=2) as vap,
            tc.tile_pool(name="pt", bufs=12) as ptp,
            tc.tile_pool(name="outs", bufs=3) as outp,
            tc.tile_pool(name="pst", bufs=3, space="PSUM") as pstp,
            tc.tile_pool(name="ppv", bufs=2, space="PSUM") as ppvp,
        ):
            # dummy exp on a 1-col tile: preloads the ACT exp table while
            # the first input DMAs are in flight
            warm = constp.tile([P, 2], F32, tag="warm")
            nc.gpsimd.memset(warm[:, 0:1], 0.0)
            nc.scalar.activation(
                warm[:, 1:2],
                warm[:, 0:1],
                mybir.ActivationFunctionType.Exp,
                scale=SCALE,
            )

            # diagonal strip masks: within each strip, col c (rel) is valid
            # iff c >= k (partition). Strips start at their diagonal.
            mask_a = constp.tile([P, 2 * SBW], BF16, tag="maskA")
            nc.gpsimd.memset(mask_a[:], 1.0)
            for _, off, w, _ in DIAG_A:
                nc.gpsimd.affine_select(
                    out=mask_a[:, off : off + w],
                    in_=mask_a[:, off : off + w],
                    compare_op=mybir.AluOpType.is_ge,
                    fill=0.0,
                    base=0,
                    pattern=[[1, w]],
                    channel_multiplier=-1,
                )
            mask_b = constp.tile([P, 256], BF16, tag="maskB")
            nc.gpsimd.memset(mask_b[:], 1.0)
            nc.gpsimd.affine_select(
                out=mask_b[:],
                in_=mask_b[:],
                compare_op=mybir.AluOpType.is_ge,
                fill=0.0,
                base=0,
                pattern=[[1, 256]],
                channel_multiplier=-1,
            )

            def pv_block(pvs, kb, j, src, qb_hi):
                # accumulate pvs[j] += src.T @ vaug for q block j
                nc.tensor.matmul(
                    pvs[j],
                    src,
                    vaug_cur[0][:, kb, :],
                    start=(kb == 0),
                    stop=(kb == qb_hi),
                )

            vaug_cur = [None]

            def head_compute(h, qt, kt_g, sb_order):
                for sb in sb_order:
                    # PV accumulators: j=0,1 live through the pair loop in 2
                    # PSUM banks (a start_tensor_calc claims a whole 2KB zero
                    # region, so groups can't share banks); j=2,3 run as a
                    # deferred burst afterwards, reusing the same 2 banks and
                    # the kept P^T tiles.
                    pvs = {
                        j: ppvp.tile(
                            [P, D + 1], F32, tag="ppv", name=f"pv_{h}_{sb}_{j}"
                        )[:]
                        for j in (0, 1)
                    }
                    kb_src = {}  # kb -> (tile, base col) for P^T slices
                    # off-diagonal pairs: kb = 2*pair, 2*pair+1, all < 4*sb
                    for pair in range(2 * sb):
                        st = pstp.tile([P, 2 * SBW], F32, tag="pst")
                        for half in (0, 1):
                            kb = 2 * pair + half
                            nc.tensor.matmul(
                                st[:, half * SBW : (half + 1) * SBW],
                                kt_g[:, kb * P : (kb + 1) * P],
                                qt[:, sb * SBW : (sb + 1) * SBW],
                                start=True,
                                stop=True,
                            )
                        pt = ptp.tile([P, 2 * SBW], BF16, tag="pt")
                        nc.scalar.activation(
                            pt[:],
                            st[:],
                            mybir.ActivationFunctionType.Exp,
                            scale=SCALE,
                        )
                        for half in (0, 1):
                            kb = 2 * pair + half
                            kb_src[kb] = (pt, half * SBW)
                            for j in (0, 1):
                                pv_block(
                                    pvs,
                                    kb,
                                    j,
                                    pt[:, half * SBW + j * P : half * SBW + (j + 1) * P],
                                    4 * sb + j,
                                )
                    # diagonal blocks kb0..kb0+3, packed valid strips
                    kb0 = 4 * sb
                    st_a = pstp.tile([P, 2 * SBW], F32, tag="pst")
                    st_b = pstp.tile([P, 2 * SBW], F32, tag="pst")
                    for dk, off, w, qoff in DIAG_A:
                        nc.tensor.matmul(
                            st_a[:, off : off + w],
                            kt_g[:, (kb0 + dk) * P : (kb0 + dk + 1) * P],
                            qt[:, sb * SBW + qoff : (sb + 1) * SBW],
                            start=True,
                            stop=True,
                        )
                    dkb, boff, bw, bqoff = DIAG_B
                    nc.tensor.matmul(
                        st_b[:, boff : boff + bw],
                        kt_g[:, (kb0 + dkb) * P : (kb0 + dkb + 1) * P],
                        qt[:, sb * SBW + bqoff : (sb + 1) * SBW],
                        start=True,
                        stop=True,
                    )
                    pt_a = ptp.tile([P, 2 * SBW], BF16, tag="pt")
                    nc.scalar.activation(
                        pt_a[:],
                        st_a[:],
                        mybir.ActivationFunctionType.Exp,
                        scale=SCALE,
                    )
                    nc.vector.tensor_tensor(
                        out=pt_a[:],
                        in0=pt_a[:],
                        in1=mask_a[:],
                        op=mybir.AluOpType.mult,
                    )
                    pt_b = ptp.tile([P, 2 * SBW], BF16, tag="pt")
                    nc.scalar.activation(
                        pt_b[:, 0:256],
                        st_b[:, 0:256],
                        mybir.ActivationFunctionType.Exp,
                        scale=SCALE,
                    )
                    nc.vector.tensor_tensor(
                        out=pt_b[:, 0:256],
                        in0=pt_b[:, 0:256],
                        in1=mask_b[:],
                        op=mybir.AluOpType.mult,
                    )
                    for dk in range(4):
                        kb = kb0 + dk
                        if dk == DIAG_B[0]:
                            _, off, w, _ = DIAG_B
                            src_t, base = pt_b, off - dk * P
                        else:
                            off, w = next(
                                (o, ww) for d, o, ww, _ in DIAG_A if d == dk
                            )
                            src_t, base = pt_a, off - dk * P
                        kb_src[kb] = (src_t, base)
                        for j in (0, 1):
                            if dk <= j:
                                pv_block(
                                    pvs,
                                    kb,
                                    j,
                                    src_t[:, base + j * P : base + (j + 1) * P],
                                    4 * sb + j,
                                )
                    ot = outp.tile([P, 4, D + 1], F32, tag="outs")
                    for j in (0, 1):
                        nc.vector.tensor_copy(ot[:, j, :], pvs[j])
                    nc.sync.dma_start(
                        o_d[h, 4 * sb : 4 * sb + 2].rearrange("qb p d -> p qb d"),
                        ot[:, 0:2, :],
                    )
                    # deferred j=2,3 accumulation burst over kept P^T tiles
                    for j in (2, 3):
                        pvs[j] = ppvp.tile(
                            [P, D + 1], F32, tag="ppv", name=f"pv_{h}_{sb}_{j}"
                        )[:]
                    for kb in range(kb0 + 4):
                        src_t, base = kb_src[kb]
                        for j in (2, 3):
                            if kb - kb0 <= j:
                                pv_block(
                                    pvs,
                                    kb,
                                    j,
                                    src_t[:, base + j * P : base + (j + 1) * P],
                                    4 * sb + j,
                                )
                    for j in (2, 3):
                        nc.vector.tensor_copy(ot[:, j, :], pvs[j])
                    nc.sync.dma_start(
                        o_d[h, 4 * sb + 2 : 4 * sb + 4].rearrange("qb p d -> p qb d"),
                        ot[:, 2:4, :],
                    )

            NCH = 4  # DMA chunks per kt/qt load (parallel queues)
            CW = SK // NCH

            def load_kt(kt_g, g, chunks):
                for ch in chunks:
                    nc.sync.dma_start(
                        kt_g[:, ch * CW : (ch + 1) * CW],
                        kt_d[g, :, ch * CW : (ch + 1) * CW],
                    )

            def load_qt(qt, h, chunks):
                for ch in chunks:
                    nc.sync.dma_start(
                        qt[:, ch * CW : (ch + 1) * CW],
                        qt_d[h, :, ch * CW : (ch + 1) * CW],
                    )

            for g in range(NKVH):
                kt_g = ktp.tile([P, SK], BF16, tag="kt")
                qt0 = qtp.tile([P, SQ], BF16, tag="qt")
                if g == 0:
                    # dispatch order follows head 0's sb order [0, 3, 2, 1]:
                    # sb=0 needs kt c0 + qt c0; sb=3 then needs kt c0-c3 and
                    # qt c3 — front-load those, defer qt c1/c2
                    load_kt(kt_g, g, [0])
                    load_qt(qt0, 2 * g, [0])
                    load_kt(kt_g, g, [1])
                    load_qt(qt0, 2 * g, [3])
                    load_kt(kt_g, g, [2, 3])
                    load_qt(qt0, 2 * g, [1, 2])
                else:
                    load_kt(kt_g, g, range(NCH))
                    load_qt(qt0, 2 * g, range(NCH))
                vaug_g = vap.tile([P, NKB, D + 1], BF16, tag="vaug")
                nc.gpsimd.memset(vaug_g[:, :, D : D + 1], 1.0)
                for ch in range(2):
                    nc.sync.dma_start(
                        vaug_g[:, ch * NKB // 2 : (ch + 1) * NKB // 2, 0:D],
                        v_d[g, :, ch * NKB // 2 : (ch + 1) * NKB // 2],
                    )
                vaug_cur[0] = vaug_g
                for hl in range(2):
                    h = 2 * g + hl
                    if hl == 0:
                        qt = qt0
                    else:
                        qt = qtp.tile([P, SQ], BF16, tag="qt")
                        load_qt(qt, h, range(NCH))
                    # end every head on a small sb so its deferred PV burst
                    # (which stalls the next head's scores on Tensor) is the
                    # smallest one; head 0 starts with sb=0 so only the first
                    # qt/kt chunks gate the pipeline start
                    sb_order = [0, 3, 2, 1] if h == 0 else [3, 2, 1, 0]
                    head_compute(h, qt[:], kt_g[:], sb_order)

    nc.finalize()
    return nc


def _get_module():
    if "nc" not in _CACHE:
        _CACHE["nc"] = build_module()
    return _CACHE["nc"]


def kernel(q, kv):
    global LAST_RESULTS
    q = np.asarray(q, dtype=np.float32)
    kv = np.asarray(kv, dtype=np.float32)

    nc = _get_module()
    in_maps = []
    for c in range(NCORES):
        b, j = divmod(c, 4)
        # qT: [Sq, 4, D] -> [4, D, Sq]
        q_s = np.ascontiguousarray(
            np.transpose(q[b][:, 4 * j : 4 * j + 4, :], (1, 2, 0))
        ).astype(ml_dtypes.bfloat16)
        # kT: [Sk, 2, D] -> [2, D, Sk]
        k_s = np.ascontiguousarray(
            np.transpose(kv[b][:, 0, 2 * j : 2 * j + 2, :], (1, 2, 0))
        ).astype(ml_dtypes.bfloat16)
        # v: [Sk, 2, D] -> [2, P, NKB, D]
        v_s = np.ascontiguousarray(
            kv[b][:, 1, 2 * j : 2 * j + 2, :]
            .reshape(NKB, P, NKVH, D)
            .transpose(2, 1, 0, 3)
        ).astype(ml_dtypes.bfloat16)
        in_maps.append({"qt": q_s, "kt": k_s, "v": v_s})

    trace = bool(int(os.environ.get("KERNEL_TRACE", "0")))
    kwargs = {}
    tdir = os.environ.get("KERNEL_TRACE_DIR")
    if tdir:
        kwargs["tmpdir"] = tdir
    res = run_bass_kernel_spmd(
        nc, in_maps, core_ids=list(range(NCORES)), trace=trace, **kwargs
    )
    LAST_RESULTS = res

    out = np.empty((B, SQ, H, D), np.float32)
    for c in range(NCORES):
        b, j = divmod(c, 4)
        o = res.results[c]["o"].reshape(NQH, SQ, D + 1)
        norm = o[..., :D] / o[..., D : D + 1]
        out[b, :, 4 * j : 4 * j + 4, :] = np.transpose(norm, (1, 0, 2))
    return out
